# revision 3
# baseline (speedup 1.0000x reference)
"""Multi-head attention (B=4, N=2048, D=768, H=12, Dh=64) on 8 TRN2 NeuronCores.

Sharding: core c -> batch b = c//2, query rows half = c%2 (1024 rows each).
Each core computes all 12 heads for its (batch, query-half) against the full
2048-key sequence, so outputs are disjoint and no collective is needed.
The per-core input xT is the batch's x transposed to [768, 2048] with the
core's query half permuted to the front (attention is permutation-invariant
over keys, so K/V over the permuted sequence give identical results).

Per-core graph:
  1. QKV^T projection: Q^T [768,1024], K^T [768,2048] (head-major partition
     tiles), V [2048, 12*65] in normal layout with a ones column per head
     (col 65h+64) so the attn@V matmul also accumulates the softmax
     denominator.
  2. Per head: S^T[k,q] = K_h^T.T @ Q_h^T, P^T = exp(0.125*S^T) (no max
     subtraction needed: scores are ~N(0,1), exp cannot overflow fp32),
     then O[q,65] = P^T.T @ [V_h | 1] accumulated over k tiles; divide by
     col 64 (denominator) into AO[q, inner].
  3. Transpose AO -> AOT [inner, q] on the PE, final projection
     out[q, d] = AOT.T @ w_out + b_out, DMA out.
"""

import numpy as np

import concourse.bass as bass
import concourse.bacc as bacc
import concourse.mybir as mybir
import concourse.tile as tile
from concourse.bass_utils import run_bass_kernel_spmd
from concourse.masks import make_identity

N_CORES = 8
B, N, D = 4, 2048, 768
H, DH = 12, 64
NQ = 1024           # query rows per core
COLS = 3 * D        # 2304 qkv columns
DT = D // 128       # 6 partition tiles of the model dim
NT = N // 128       # 16 key tiles
QT_TILES = NQ // 128  # 8 query tiles
VG = DH + 1         # 65: head group width in V (64 cols + ones)

F32 = mybir.dt.float32
BF16 = mybir.dt.bfloat16

# dtype knobs
PROJ_CAST = None      # None = f32 matmuls for projection; or mybir.dt.float32r


def _mm_dt(ap):
    """Apply the projection dtype cast knob to an f32 AP."""
    if PROJ_CAST is not None:
        return ap.bitcast(PROJ_CAST)
    return ap


def build():
    nc = bacc.Bacc("TRN2", target_bir_lowering=False, debug=False,
                   num_devices=N_CORES)

    xT_d = nc.dram_tensor("xT", [D, N], F32, kind="ExternalInput")
    wqkv_d = nc.dram_tensor("wqkv", [D, COLS], F32, kind="ExternalInput")
    wout_d = nc.dram_tensor("wout", [D, D], F32, kind="ExternalInput")
    bias_d = nc.dram_tensor("bias", [128, D], F32, kind="ExternalInput")
    out_d = nc.dram_tensor("out", [NQ, D], F32, kind="ExternalOutput")

    with tile.TileContext(nc) as tc:
        with tc.tile_pool(name="persist", bufs=1) as pp, \
             tc.tile_pool(name="small", bufs=1) as smallp, \
             tc.tile_pool(name="outs", bufs=4) as outsp:

            # ---- persistent tiles ----
            QT = [pp.tile([128, NQ], BF16, name=f"QT{i}", tag=f"QT{i}")
                  for i in range(DT)]
            KT = [pp.tile([128, N], BF16, name=f"KT{i}", tag=f"KT{i}")
                  for i in range(DT)]
            V = [pp.tile([128, H * VG], BF16, name=f"V{i}", tag=f"V{i}")
                 for i in range(NT)]
            AO = [pp.tile([128, D], BF16, name=f"AO{i}", tag=f"AO{i}")
                  for i in range(QT_TILES)]
            AOT = [pp.tile([128, NQ], BF16, name=f"AOT{i}", tag=f"AOT{i}")
                   for i in range(DT)]
            WO = [pp.tile([128, D], BF16, name=f"WO{i}", tag=f"WO{i}")
                  for i in range(DT)]
            BIAS = pp.tile([128, D], F32, name="BIAS", tag="BIAS")
            IDENT = pp.tile([128, 128], BF16, name="IDENT", tag="IDENT")

            make_identity(nc, IDENT[:])
            nc.sync.dma_start(BIAS[:], bias_d.ap())

            # w_out: load f32, cast to bf16
            for i in range(DT):
                wtmp = smallp.tile([128, D], F32, name="wtmp", tag="wtmp",
                                   bufs=2)
                nc.sync.dma_start(wtmp[:], wout_d.ap()[i * 128:(i + 1) * 128, :])
                nc.vector.tensor_copy(WO[i][:], wtmp[:])

            # ---- phase A: load x^T, stream W panels; project QKV ----
            with tc.tile_pool(name="projin", bufs=1) as projin, \
                 tc.tile_pool(name="psA", bufs=4, space="PSUM") as psA:
                xT = [projin.tile([128, N], F32, name=f"xT{i}", tag=f"xT{i}")
                      for i in range(DT)]
                for i in range(DT):
                    nc.sync.dma_start(xT[i][:],
                                      xT_d.ap()[i * 128:(i + 1) * 128, :])

                # Q^T [768, 1024] (panel i = w cols 128i..) and
                # K^T [768, 2048] (panel 6+i = w cols 768+128i..)
                for pi in range(2 * DT):
                    co = pi * 128
                    wp = [projin.tile([128, 128], F32, name=f"wqk{d}",
                                      tag=f"wqk{d}", bufs=2)
                          for d in range(DT)]
                    for d in range(DT):
                        nc.sync.dma_start(
                            wp[d][:],
                            wqkv_d.ap()[d * 128:(d + 1) * 128, co:co + 128])
                    is_q = pi < DT
                    dst = QT[pi] if is_q else KT[pi - DT]
                    nn = NQ if is_q else N
                    for nb in range(nn // 512):
                        ps = psA.tile([128, 512], F32, name="psA", tag="psA")
                        for d in range(DT):
                            nc.tensor.matmul(
                                ps[:],
                                _mm_dt(wp[d][:]),
                                _mm_dt(xT[d][:, nb * 512:(nb + 1) * 512]),
                                start=(d == 0), stop=(d == DT - 1))
                        nc.vector.tensor_copy(
                            dst[:, nb * 512:(nb + 1) * 512], ps[:])

                # V [2048, 12*65]: normal layout, x^T as stationary
                for vp in range(3):  # panels of 256 v-cols = 4 heads
                    co = 2 * D + vp * 256
                    wv = [projin.tile([128, 256], F32, name=f"wv{d}",
                                      tag=f"wv{d}", bufs=2)
                          for d in range(DT)]
                    for d in range(DT):
                        nc.sync.dma_start(
                            wv[d][:],
                            wqkv_d.ap()[d * 128:(d + 1) * 128, co:co + 256])
                    for t in range(NT):
                        ps = psA.tile([128, 512], F32, name="psA", tag="psA")
                        for d in range(DT):
                            nc.tensor.matmul(
                                ps[:, :256],
                                _mm_dt(xT[d][:, t * 128:(t + 1) * 128]),
                                _mm_dt(wv[d][:]),
                                start=(d == 0), stop=(d == DT - 1))
                        dst = V[t][:].rearrange("p (h c) -> p h c", c=VG)
                        nc.vector.tensor_copy(
                            dst[:, vp * 4:(vp + 1) * 4, 0:DH],
                            ps[:, :256].rearrange("p (h c) -> p h c", c=DH))
                for t in range(NT):
                    ones = V[t][:].rearrange("p (h c) -> p h c",
                                             c=VG)[:, :, DH:VG]
                    nc.gpsimd.memset(ones, 1.0)

            # ---- phase B: attention per head ----
            with tc.tile_pool(name="pt", bufs=1) as ptp, \
                 tc.tile_pool(name="psS", bufs=3, space="PSUM") as psS, \
                 tc.tile_pool(name="psO", bufs=4, space="PSUM") as psO:
                for h in range(H):
                    ht, hp = divmod(h, 2)
                    kt_h = KT[ht][hp * DH:(hp + 1) * DH, :]
                    qt_h = QT[ht][hp * DH:(hp + 1) * DH, :]
                    for qb in range(NQ // 512):
                        PT = [ptp.tile([128, 512], BF16, name=f"PT{k}",
                                       tag=f"PT{k}", bufs=2)
                              for k in range(NT)]
                        for k in range(NT):
                            ps = psS.tile([128, 512], F32, name="psS",
                                          tag="psS")
                            nc.tensor.matmul(
                                ps[:],
                                kt_h[:, k * 128:(k + 1) * 128],
                                qt_h[:, qb * 512:(qb + 1) * 512],
                                start=True, stop=True)
                            nc.scalar.activation(
                                PT[k][:], ps[:],
                                mybir.ActivationFunctionType.Exp, scale=0.125)
                        for qi in range(4):
                            qt = qb * 4 + qi
                            po = psO.tile([128, VG], F32, name="psO",
                                          tag="psO")
                            for k in range(NT):
                                nc.tensor.matmul(
                                    po[:],
                                    PT[k][:, qi * 128:(qi + 1) * 128],
                                    V[k][:, h * VG:(h + 1) * VG],
                                    start=(k == 0), stop=(k == NT - 1))
                            rc = smallp.tile([128, 1], F32, name="rc",
                                             tag="rc", bufs=4)
                            nc.vector.reciprocal(rc[:], po[:, DH:VG])
                            nc.vector.tensor_scalar_mul(
                                AO[qt][:, h * DH:(h + 1) * DH],
                                po[:, 0:DH], rc[:])

            # ---- phase C: transpose AO, final projection ----
            with tc.tile_pool(name="psC", bufs=1, space="PSUM") as psC:
                for i in range(DT):
                    for qt in range(QT_TILES):
                        pt_ = psC.tile([128, 128], BF16, name="psT",
                                       tag="psT", bufs=2)
                        nc.tensor.transpose(
                            pt_[:], AO[qt][:, i * 128:(i + 1) * 128],
                            IDENT[:])
                        nc.vector.tensor_copy(
                            AOT[i][:, qt * 128:(qt + 1) * 128], pt_[:])
                for qt in range(QT_TILES):
                    for (fo, fsz) in ((0, 512), (512, 256)):
                        ps = psC.tile([128, 512], F32, name="psF", tag="psF",
                                      bufs=3)
                        for i in range(DT):
                            nc.tensor.matmul(
                                ps[:, :fsz],
                                AOT[i][:, qt * 128:(qt + 1) * 128],
                                WO[i][:, fo:fo + fsz],
                                start=(i == 0), stop=(i == DT - 1))
                        ot = outsp.tile([128, 512], F32, name="ot", tag="ot")
                        nc.vector.tensor_add(
                            ot[:, :fsz], ps[:, :fsz], BIAS[:, fo:fo + fsz])
                        nc.sync.dma_start(
                            out_d.ap()[qt * 128:(qt + 1) * 128, fo:fo + fsz],
                            ot[:, :fsz])

    nc.compile()
    return nc


_NC = None


def _get_nc():
    global _NC
    if _NC is None:
        _NC = build()
    return _NC


def make_in_maps(x, w_qkv, w_out, b_out):
    x = np.asarray(x, np.float32)
    w_qkv = np.ascontiguousarray(np.asarray(w_qkv, np.float32))
    w_out = np.ascontiguousarray(np.asarray(w_out, np.float32))
    bias = np.ascontiguousarray(
        np.broadcast_to(np.asarray(b_out, np.float32)[None, :], (128, D)))
    in_maps = []
    for c in range(N_CORES):
        b, half = divmod(c, 2)
        xb = x[b]
        qoff = half * NQ
        # query half first; key order permutation is harmless
        xperm = np.vstack([xb[qoff:qoff + NQ], xb[NQ - qoff:2 * NQ - qoff]])
        in_maps.append({
            "xT": np.ascontiguousarray(xperm.T),
            "wqkv": w_qkv,
            "wout": w_out,
            "bias": bias,
        })
    return in_maps


def run(in_maps, trace=False, **kw):
    return run_bass_kernel_spmd(_get_nc(), in_maps,
                                core_ids=list(range(N_CORES)),
                                trace=trace, **kw)


def assemble(results):
    out = np.empty((B, N, D), np.float32)
    for c in range(N_CORES):
        b, half = divmod(c, 2)
        out[b, half * NQ:(half + 1) * NQ, :] = results[c]["out"]
    return out


def kernel(x, w_qkv, w_out, b_out):
    res = run(make_in_maps(x, w_qkv, w_out, b_out))
    return assemble(res.results)


# revision 5
# speedup vs baseline: 1.4801x; 1.4801x over previous
"""Multi-head attention (B=4, N=2048, D=768, H=12, Dh=64) on 8 TRN2 NeuronCores.

Sharding: core c -> batch b = c//2, query rows half = c%2 (1024 rows each).
Each core computes all 12 heads for its (batch, query-half) against the full
2048-key sequence, so outputs are disjoint and no collective is needed.
The per-core input xT is the batch's x transposed to [768, 2048] with the
core's query half permuted to the front (attention is permutation-invariant
over keys, so K/V over the permuted sequence give identical results).

Per-core graph:
  1. QKV^T projection: Q^T [768,1024], K^T [768,2048] (head-major partition
     tiles), V [2048, 12*65] in normal layout with a ones column per head
     (col 65h+64) so the attn@V matmul also accumulates the softmax
     denominator.
  2. Per head: S^T[k,q] = K_h^T.T @ Q_h^T, P^T = exp(0.125*S^T) (no max
     subtraction needed: scores are ~N(0,1), exp cannot overflow fp32),
     then O[q,65] = P^T.T @ [V_h | 1] accumulated over k tiles; divide by
     col 64 (denominator) into AO[q, inner].
  3. Transpose AO -> AOT [inner, q] on the PE, final projection
     out[q, d] = AOT.T @ w_out + b_out, DMA out.
"""

import numpy as np

import concourse.bass as bass
import concourse.bacc as bacc
import concourse.mybir as mybir
import concourse.tile as tile
from concourse.bass_utils import run_bass_kernel_spmd
from concourse.masks import make_identity

N_CORES = 8
B, N, D = 4, 2048, 768
H, DH = 12, 64
NQ = 1024           # query rows per core
COLS = 3 * D        # 2304 qkv columns
DT = D // 128       # 6 partition tiles of the model dim
NT = N // 128       # 16 key tiles
QT_TILES = NQ // 128  # 8 query tiles
VG = DH + 1         # 65: head group width in V (64 cols + ones)

F32 = mybir.dt.float32
BF16 = mybir.dt.bfloat16

# dtype knobs
PROJ_CAST = None  # unused: projection runs in bf16


def _mm_dt(ap):
    """Apply the projection dtype cast knob to an f32 AP."""
    if PROJ_CAST is not None:
        return ap.bitcast(PROJ_CAST)
    return ap


def build():
    nc = bacc.Bacc("TRN2", target_bir_lowering=False, debug=False,
                   num_devices=N_CORES)

    xT_d = nc.dram_tensor("xT", [D, N], BF16, kind="ExternalInput")
    wqkv_d = nc.dram_tensor("wqkv", [D, COLS], BF16, kind="ExternalInput")
    wout_d = nc.dram_tensor("wout", [D, D], BF16, kind="ExternalInput")
    bias_d = nc.dram_tensor("bias", [128, D], F32, kind="ExternalInput")
    out_d = nc.dram_tensor("out", [NQ, D], F32, kind="ExternalOutput")

    with tile.TileContext(nc) as tc:
        with tc.tile_pool(name="persist", bufs=1) as pp, \
             tc.tile_pool(name="small", bufs=1) as smallp, \
             tc.tile_pool(name="outs", bufs=4) as outsp:

            # ---- persistent tiles ----
            QT = [pp.tile([128, NQ], BF16, name=f"QT{i}", tag=f"QT{i}")
                  for i in range(DT)]
            KT = [pp.tile([128, N], BF16, name=f"KT{i}", tag=f"KT{i}")
                  for i in range(DT)]
            V = [pp.tile([128, H * VG], BF16, name=f"V{i}", tag=f"V{i}")
                 for i in range(NT)]
            AO = [pp.tile([128, D], BF16, name=f"AO{i}", tag=f"AO{i}")
                  for i in range(QT_TILES)]
            AOT = [pp.tile([128, NQ], BF16, name=f"AOT{i}", tag=f"AOT{i}")
                   for i in range(DT)]
            WO = [pp.tile([128, D], BF16, name=f"WO{i}", tag=f"WO{i}")
                  for i in range(DT)]
            BIAS = pp.tile([128, D], F32, name="BIAS", tag="BIAS")
            IDENT = pp.tile([128, 128], BF16, name="IDENT", tag="IDENT")

            make_identity(nc, IDENT[:])
            nc.sync.dma_start(BIAS[:], bias_d.ap())

            for i in range(DT):
                nc.sync.dma_start(WO[i][:], wout_d.ap()[i * 128:(i + 1) * 128, :])

            # ---- phase A: load x^T, stream W panels; project QKV ----
            with tc.tile_pool(name="projin", bufs=1) as projin, \
                 tc.tile_pool(name="psA", bufs=4, space="PSUM") as psA:
                xT = [projin.tile([128, N], BF16, name=f"xT{i}", tag=f"xT{i}")
                      for i in range(DT)]
                for i in range(DT):
                    nc.sync.dma_start(xT[i][:],
                                      xT_d.ap()[i * 128:(i + 1) * 128, :])

                # Q^T [768, 1024] (panel i = w cols 128i..) and
                # K^T [768, 2048] (panel 6+i = w cols 768+128i..)
                for pi in range(2 * DT):
                    co = pi * 128
                    wp = [projin.tile([128, 128], BF16, name=f"wqk{d}",
                                      tag=f"wqk{d}", bufs=2)
                          for d in range(DT)]
                    for d in range(DT):
                        nc.sync.dma_start(
                            wp[d][:],
                            wqkv_d.ap()[d * 128:(d + 1) * 128, co:co + 128])
                    is_q = pi < DT
                    dst = QT[pi] if is_q else KT[pi - DT]
                    nn = NQ if is_q else N
                    for nb in range(nn // 512):
                        ps = psA.tile([128, 512], F32, name="psA", tag="psA")
                        for d in range(DT):
                            nc.tensor.matmul(
                                ps[:],
                                _mm_dt(wp[d][:]),
                                _mm_dt(xT[d][:, nb * 512:(nb + 1) * 512]),
                                start=(d == 0), stop=(d == DT - 1))
                        nc.vector.tensor_copy(
                            dst[:, nb * 512:(nb + 1) * 512], ps[:])

                # V [2048, 12*65]: normal layout, x^T as stationary
                for vp in range(3):  # panels of 256 v-cols = 4 heads
                    co = 2 * D + vp * 256
                    wv = [projin.tile([128, 256], BF16, name=f"wv{d}",
                                      tag=f"wv{d}", bufs=2)
                          for d in range(DT)]
                    for d in range(DT):
                        nc.sync.dma_start(
                            wv[d][:],
                            wqkv_d.ap()[d * 128:(d + 1) * 128, co:co + 256])
                    for t in range(NT):
                        ps = psA.tile([128, 512], F32, name="psA", tag="psA")
                        for d in range(DT):
                            nc.tensor.matmul(
                                ps[:, :256],
                                _mm_dt(xT[d][:, t * 128:(t + 1) * 128]),
                                _mm_dt(wv[d][:]),
                                start=(d == 0), stop=(d == DT - 1))
                        dst = V[t][:].rearrange("p (h c) -> p h c", c=VG)
                        nc.vector.tensor_copy(
                            dst[:, vp * 4:(vp + 1) * 4, 0:DH],
                            ps[:, :256].rearrange("p (h c) -> p h c", c=DH))
                for t in range(NT):
                    ones = V[t][:].rearrange("p (h c) -> p h c",
                                             c=VG)[:, :, DH:VG]
                    nc.gpsimd.memset(ones, 1.0)

            # ---- phase B: attention per head ----
            with tc.tile_pool(name="pt", bufs=1) as ptp, \
                 tc.tile_pool(name="psS", bufs=3, space="PSUM") as psS, \
                 tc.tile_pool(name="psO", bufs=4, space="PSUM") as psO:
                for h in range(H):
                    ht, hp = divmod(h, 2)
                    kt_h = KT[ht][hp * DH:(hp + 1) * DH, :]
                    qt_h = QT[ht][hp * DH:(hp + 1) * DH, :]
                    for qb in range(NQ // 512):
                        PT = [ptp.tile([128, 512], BF16, name=f"PT{k}",
                                       tag=f"PT{k}", bufs=2)
                              for k in range(NT)]
                        for k in range(NT):
                            ps = psS.tile([128, 512], F32, name="psS",
                                          tag="psS")
                            nc.tensor.matmul(
                                ps[:],
                                kt_h[:, k * 128:(k + 1) * 128],
                                qt_h[:, qb * 512:(qb + 1) * 512],
                                start=True, stop=True)
                            nc.scalar.activation(
                                PT[k][:], ps[:],
                                mybir.ActivationFunctionType.Exp, scale=0.125)
                        for qi in range(4):
                            qt = qb * 4 + qi
                            po = psO.tile([128, VG], F32, name="psO",
                                          tag="psO")
                            for k in range(NT):
                                nc.tensor.matmul(
                                    po[:],
                                    PT[k][:, qi * 128:(qi + 1) * 128],
                                    V[k][:, h * VG:(h + 1) * VG],
                                    start=(k == 0), stop=(k == NT - 1))
                            rc = smallp.tile([128, 1], F32, name="rc",
                                             tag="rc", bufs=4)
                            nc.vector.reciprocal(rc[:], po[:, DH:VG])
                            nc.vector.tensor_scalar_mul(
                                AO[qt][:, h * DH:(h + 1) * DH],
                                po[:, 0:DH], rc[:])

            # ---- phase C: transpose AO, final projection ----
            with tc.tile_pool(name="psC", bufs=1, space="PSUM") as psC:
                for i in range(DT):
                    for qt in range(QT_TILES):
                        pt_ = psC.tile([128, 128], BF16, name="psT",
                                       tag="psT", bufs=2)
                        nc.tensor.transpose(
                            pt_[:], AO[qt][:, i * 128:(i + 1) * 128],
                            IDENT[:])
                        nc.vector.tensor_copy(
                            AOT[i][:, qt * 128:(qt + 1) * 128], pt_[:])
                for qt in range(QT_TILES):
                    for (fo, fsz) in ((0, 512), (512, 256)):
                        ps = psC.tile([128, 512], F32, name="psF", tag="psF",
                                      bufs=3)
                        for i in range(DT):
                            nc.tensor.matmul(
                                ps[:, :fsz],
                                AOT[i][:, qt * 128:(qt + 1) * 128],
                                WO[i][:, fo:fo + fsz],
                                start=(i == 0), stop=(i == DT - 1))
                        ot = outsp.tile([128, 512], F32, name="ot", tag="ot")
                        nc.vector.tensor_add(
                            ot[:, :fsz], ps[:, :fsz], BIAS[:, fo:fo + fsz])
                        nc.sync.dma_start(
                            out_d.ap()[qt * 128:(qt + 1) * 128, fo:fo + fsz],
                            ot[:, :fsz])

    nc.compile()
    return nc


_NC = None


def _get_nc():
    global _NC
    if _NC is None:
        _NC = build()
    return _NC


def make_in_maps(x, w_qkv, w_out, b_out):
    import ml_dtypes
    x = np.asarray(x, np.float32)
    w_qkv = np.ascontiguousarray(np.asarray(w_qkv, ml_dtypes.bfloat16))
    w_out = np.ascontiguousarray(np.asarray(w_out, ml_dtypes.bfloat16))
    bias = np.ascontiguousarray(
        np.broadcast_to(np.asarray(b_out, np.float32)[None, :], (128, D)))
    in_maps = []
    for c in range(N_CORES):
        b, half = divmod(c, 2)
        xb = x[b]
        qoff = half * NQ
        # query half first; key order permutation is harmless
        xperm = np.vstack([xb[qoff:qoff + NQ], xb[NQ - qoff:2 * NQ - qoff]])
        in_maps.append({
            "xT": np.ascontiguousarray(xperm.T.astype(ml_dtypes.bfloat16)),
            "wqkv": w_qkv,
            "wout": w_out,
            "bias": bias,
        })
    return in_maps


def run(in_maps, trace=False, **kw):
    return run_bass_kernel_spmd(_get_nc(), in_maps,
                                core_ids=list(range(N_CORES)),
                                trace=trace, **kw)


def assemble(results):
    out = np.empty((B, N, D), np.float32)
    for c in range(N_CORES):
        b, half = divmod(c, 2)
        out[b, half * NQ:(half + 1) * NQ, :] = results[c]["out"]
    return out


def kernel(x, w_qkv, w_out, b_out):
    res = run(make_in_maps(x, w_qkv, w_out, b_out))
    return assemble(res.results)
